# revision 1
# baseline (speedup 1.0000x reference)
"""Causal self-attention Bass/Tile kernel for Trainium2, 8 NeuronCores.

Problem: B=4, T=2048, C=1024, NH=16, HD=64.
  q/k/v = x @ W{q,k,v}; att = softmax(causal(q k^T / 8)); y = (att v) @ Wp

Sharding (8 cores): batch (4-way) x head-group (2-way tensor parallel).
Core c handles batch b=c//2 and global heads g*8..g*8+7 where g=c%2.
Each core computes a partial projection y_part = y_heads_local @ Wp[rows]
and the host unshards by summing the two partial outputs per batch.

Per-core kernel (all T=2048 tokens, 8 heads, head_dim 64), bf16 matmuls
with fp32 PSUM accumulation and fp32 softmax:
  Phase A: x^T, Wq, Wk, Wv resident in SBUF (bf16); qT/kT = (x W)^T
           stored [d, t], v stored [t, d] augmented with a ones column so
           P^T@[V|1] also yields the softmax denominator l in PSUM row 64.
  Phase B: per query tile j / head pair: transposed score tiles
           S^T [s:128, t:512] on PE with the two heads of the pair on
           disjoint PE row halves (concurrent sub-array execution),
           exp(S/8) on ACT (PSUM->SBUF bf16), causal mask via GPSIMD
           affine_select (fill 0 post-exp), P^T@[V|1] accumulating
           unnormalized out^T [65, t] per head in PSUM.
  Phase C: l -> 1/l (DVE reciprocal), pair-broadcast over 128 head dims
           via a K=2 fp32 selector matmul, normalize out^T (DVE multiply).
  Phase D: y_part[t, c] = sum_u ylocT[u, t] * Wp[u, c] on PE (bf16).
"""

import numpy as np

B, T, C, NH, HD = 4, 2048, 1024, 16, 64
G = 512          # local head dims per core (8 heads x 64)
P = 128
NT = 4           # t tiles of 512
NT128 = 16       # t tiles of 128
NPAIR = 4        # local head pairs
TT = 512

_CACHE = {}


def _build_nc():
    import concourse.tile as tile
    from concourse import bacc, mybir

    f32 = mybir.dt.float32
    bf16 = mybir.dt.bfloat16

    nc = bacc.Bacc("TRN2", target_bir_lowering=False, debug=False)

    xT = nc.dram_tensor("xt", [C, T], bf16, kind="ExternalInput")
    wq = nc.dram_tensor("wq", [C, G], bf16, kind="ExternalInput")
    wk = nc.dram_tensor("wk", [C, G], bf16, kind="ExternalInput")
    wv = nc.dram_tensor("wv", [C, G], bf16, kind="ExternalInput")
    wp = nc.dram_tensor("wp", [G, C], bf16, kind="ExternalInput")
    sel = nc.dram_tensor("sel", [2, P], f32, kind="ExternalInput")
    y = nc.dram_tensor("y", [T, C], f32, kind="ExternalOutput")

    xT_v = xT.rearrange("(co p) t -> p co t", p=P)      # [128, 8, 2048]
    wq_v = wq.rearrange("(co p) g -> p co g", p=P)      # [128, 8, 512]
    wk_v = wk.rearrange("(co p) g -> p co g", p=P)
    wv_v = wv.rearrange("(co p) g -> p co g", p=P)
    wp_v = wp.rearrange("(uo p) c -> p uo c", p=P)      # [128, 4, 1024]
    y_v = y.rearrange("(to p) c -> p to c", p=P)        # [128, 16, 1024]

    with tile.TileContext(nc) as tc:
        with (
            tc.tile_pool(name="singles", bufs=1) as singles,
            tc.tile_pool(name="expst", bufs=2) as epool,
            tc.tile_pool(name="bcast", bufs=1) as bpool,
            tc.tile_pool(name="rf", bufs=2) as rfpool,
            tc.tile_pool(name="ystage", bufs=3) as ypool,
            tc.tile_pool(name="psA", bufs=4, space="PSUM") as psA,
            tc.tile_pool(name="psS", bufs=2, space="PSUM") as psS,
        ):
            # persistent tensors
            xT_sb = singles.tile([P, 8, T], bf16, name="xT_sb", tag="xT_sb")
            # wqk_sb[:, co, 2*dg+view, :]: lhsT tiles for q (view 0), k (view 1)
            wqk_sb = singles.tile([P, 8, 8, P], bf16, name="wqk_sb", tag="wqk_sb")
            wv_sb = singles.tile([P, 8, G], bf16, name="wv_sb", tag="wv_sb")
            wp_sb = singles.tile([P, NPAIR, C], bf16, name="wp_sb", tag="wp_sb")
            qT = singles.tile([P, NPAIR, T], bf16, name="qT", tag="qT")
            kT = singles.tile([P, NPAIR, T], bf16, name="kT", tag="kT")
            v_sb = singles.tile([P, NT128, 8, 66], bf16, name="v_sb", tag="v_sb")
            ylocT = singles.tile([P, NPAIR, T], bf16, name="ylocT", tag="ylocT")
            # l for (h, j) lives at partition 32*j, free slot h (DVE copies
            # out of PSUM row 64 may only target partitions 0/32/64/96);
            # l8 holds pair pr at partitions {32pr, 32pr+1}
            lq = singles.tile([P, 8, TT], f32, name="lq", tag="lq")
            l8 = singles.tile([P, NT, TT], f32, name="l8", tag="l8")
            sel_sb = singles.tile([2, P], f32, name="sel_sb", tag="sel_sb")

            nc.vector.memset(v_sb[:, :, :, 64:65], 1.0)
            nc.vector.memset(l8[:], 1.0)
            nc.gpsimd.dma_start(sel_sb[:], sel[:])
            # load order: dg0 weights first so PE starts ~immediately, then
            # x chunks; remaining weights ride other engines' DMA queues.
            nc.sync.dma_start(wqk_sb[:, :, 0, :], wq_v[:, :, 0:P])
            nc.scalar.dma_start(wqk_sb[:, :, 1, :], wk_v[:, :, 0:P])
            for th in range(2):
                for co in range(8):
                    nc.sync.dma_start(
                        xT_sb[:, co, th * 1024:(th + 1) * 1024],
                        xT_v[:, co, th * 1024:(th + 1) * 1024])
            for dg in range(1, NPAIR):
                for view, w_view in ((0, wq_v), (1, wk_v)):
                    nc.scalar.dma_start(
                        wqk_sb[:, :, 2 * dg + view, :],
                        w_view[:, :, dg * P:(dg + 1) * P])
            nc.scalar.dma_start(wv_sb[:], wv_v[:])
            nc.gpsimd.dma_start(wp_sb[:], wp_v[:])

            # ----- Phases A (projections) and B (attention), interleaved -----
            # A's PE-dense blocks are emitted between B's ACT-paced blocks so
            # the scheduler can fill PE idle time while ACT streams exps.
            def emit_A(dg):
                for view, dstT in ((0, qT), (1, kT)):
                    for jj in range(NT):
                        ps = psA.tile([P, TT], f32, name="ps_qk", tag="psA")
                        for co in range(8):
                            nc.tensor.matmul(
                                ps[:], wqk_sb[:, co, 2 * dg + view, :],
                                xT_sb[:, co, jj * TT:(jj + 1) * TT],
                                start=(co == 0), stop=(co == 7))
                        nc.vector.tensor_copy(
                            out=dstT[:, dg, jj * TT:(jj + 1) * TT], in_=ps[:])
                for tq in range(4):
                    t128 = 4 * dg + tq
                    ps = psA.tile([P, G], f32, name="ps_v", tag="psA")
                    for co in range(8):
                        nc.tensor.matmul(
                            ps[:], xT_sb[:, co, t128 * P:(t128 + 1) * P],
                            wv_sb[:, co, :],
                            start=(co == 0), stop=(co == 7))
                    nc.vector.tensor_copy(
                        out=v_sb[:, t128, :, 0:64],
                        in_=ps.rearrange("p (h d) -> p h d", h=8))

            def emit_B(j, pr):
                ns = 4 * (j + 1)  # s tiles of 128 in causal prefix
                # diagonal s-tiles first so the GPSIMD mask overlaps the
                # remaining QK/exp stream and PV can start early
                so_order = list(range(4 * j, 4 * j + 4)) + list(range(4 * j))
                expp_lo = epool.tile(
                    [P, 8, 2, TT], bf16, name="expp_lo", tag="expp")
                expp_hi = expp_lo if ns <= 8 else epool.tile(
                    [P, 8, 2, TT], bf16, name="expp_hi", tag="expp")

                def eslc(so, hi_, _lo=expp_lo, _hi=expp_hi):
                    t = _lo if so < 8 else _hi
                    return t[:, so % 8, hi_, :]

                def eslc4(lo4, hi_, _lo=expp_lo, _hi=expp_hi):
                    t = _lo if lo4 < 8 else _hi
                    return t[:, lo4 % 8:lo4 % 8 + 4, hi_, :]
                for si, so in enumerate(so_order):
                    ps_s = psS.tile([P, 2, TT], f32, name="ps_s", tag="psS")
                    for hi in range(2):
                        hp = 64 * hi
                        nc.tensor.matmul(
                            ps_s[:, hi, :],
                            kT[hp:hp + 64, pr, so * P:(so + 1) * P],
                            qT[hp:hp + 64, pr, j * TT:(j + 1) * TT],
                            start=True, stop=True)
                    nc.scalar.activation(
                        out=(expp_lo if so < 8 else expp_hi)[:, so % 8, :, :],
                        in_=ps_s[:],
                        func=mybir.ActivationFunctionType.Exp,
                        scale=0.125)
                    if si == 3:
                        # causal mask on diagonal 4 s-tiles (s > t -> 0)
                        for hi in range(2):
                            nc.gpsimd.affine_select(
                                out=eslc4(4 * j, hi),
                                in_=eslc4(4 * j, hi),
                                pattern=[[-P, 4], [1, TT]],
                                compare_op=mybir.AluOpType.is_ge,
                                fill=0.0,
                                base=0,
                                channel_multiplier=-1)
                # P^T @ [v | 1] accumulating out^T (65 rows) per head
                for hi in range(2):
                    h = 2 * pr + hi
                    hp = 64 * hi
                    ps_o = psA.tile([P, TT], f32, name="ps_o", tag="psA")
                    for si, so in enumerate(so_order):
                        nc.tensor.matmul(
                            ps_o[0:65, :],
                            v_sb[:, so, h, 0:65],
                            eslc(so, hi),
                            start=(si == 0), stop=(si == ns - 1))
                    nc.vector.tensor_copy(
                        out=ylocT[hp:hp + 64, pr, j * TT:(j + 1) * TT],
                        in_=ps_o[0:64, :])
                    nc.vector.tensor_copy(
                        out=lq[32 * j:32 * j + 1, h, :],
                        in_=ps_o[64:65, :])

            for dg in range(NPAIR):
                emit_A(dg)
            for j in range(NT):
                for pr in range(NPAIR):
                    emit_B(j, pr)

            # ---------------- Phase C: normalize ----------------
            # hc-major so phase D's first half unblocks while hc=1 runs
            for hc in range(2):
                for jj in range(2):
                    j = 2 * hc + jj
                    for pr in range(NPAIR):
                        nc.sync.dma_start(
                            out=l8[32 * pr:32 * pr + 2, j, :],
                            in_=lq[32 * j:32 * j + 1, 2 * pr:2 * pr + 2, :])
                nc.vector.reciprocal(
                    out=l8[:, 2 * hc:2 * hc + 2, :],
                    in_=l8[:, 2 * hc:2 * hc + 2, :])
                # bcast[m, t] = sel[0, m]*recip_h0[t] + sel[1, m]*recip_h1[t]
                for pr in range(NPAIR):
                    rf = rfpool.tile([2, 2, TT], f32, name="rf", tag="rf")
                    nc.sync.dma_start(
                        out=rf[:],
                        in_=l8[32 * pr:32 * pr + 2, 2 * hc:2 * hc + 2, :])
                    ps_b = psS.tile([P, 2, TT], f32, name="ps_b", tag="psS")
                    for u in range(2):
                        nc.tensor.matmul(
                            ps_b[:, u, :], sel_sb[:], rf[:, u, :],
                            start=True, stop=True)
                    bc = bpool.tile([P, 2, TT], f32, name="bc", tag="bc")
                    nc.vector.tensor_copy(out=bc[:], in_=ps_b[:])
                    yv = ylocT[:, pr, hc * 1024:(hc + 1) * 1024]
                    nc.vector.tensor_tensor(
                        out=yv.rearrange("p (a b) -> p a b", a=2),
                        in0=yv.rearrange("p (a b) -> p a b", a=2),
                        in1=bc[:],
                        op=mybir.AluOpType.mult)

            # ---------------- Phase D: output projection ----------------
            for t128 in range(NT128):
                for cn in range(2):
                    ps_y = psA.tile([P, TT], f32, name="ps_y", tag="psA")
                    for uo in range(4):
                        nc.tensor.matmul(
                            ps_y[:],
                            ylocT[:, uo, t128 * P:(t128 + 1) * P],
                            wp_sb[:, uo, cn * TT:(cn + 1) * TT],
                            start=(uo == 0), stop=(uo == 3))
                    yst = ypool.tile([P, TT], f32, name="yst", tag="yst")
                    nc.vector.tensor_copy(out=yst[:], in_=ps_y[:])
                    nc.sync.dma_start(
                        out=y_v[:, t128, cn * TT:(cn + 1) * TT],
                        in_=yst[:])

    nc.finalize()
    return nc


def _get_nc():
    if "nc" not in _CACHE:
        _CACHE["nc"] = _build_nc()
    return _CACHE["nc"]


def _sel_array():
    sel = np.zeros((2, P), np.float32)
    sel[0, 0:64] = 1.0
    sel[1, 64:128] = 1.0
    return sel


def shard_inputs(x, Wq, Wk, Wv, Wp):
    """Build the 8 per-core input maps."""
    import ml_dtypes
    bf = ml_dtypes.bfloat16
    x = np.asarray(x, np.float32)
    Wq, Wk, Wv, Wp = (np.asarray(w, np.float32) for w in (Wq, Wk, Wv, Wp))
    in_maps = []
    for c in range(8):
        b, g = c // 2, c % 2
        sl = slice(g * G, (g + 1) * G)
        in_maps.append({
            "xt": np.ascontiguousarray(x[b].T).astype(bf),
            "wq": np.ascontiguousarray(Wq[:, sl]).astype(bf),
            "wk": np.ascontiguousarray(Wk[:, sl]).astype(bf),
            "wv": np.ascontiguousarray(Wv[:, sl]).astype(bf),
            "wp": np.ascontiguousarray(Wp[sl, :]).astype(bf),
            "sel": _sel_array(),
        })
    return in_maps


def unshard_outputs(results):
    """results: list of 8 dicts with 'y' [T, C] partials -> [B, T, C]."""
    out = np.empty((B, T, C), np.float32)
    for b in range(B):
        out[b] = results[2 * b]["y"] + results[2 * b + 1]["y"]
    return out


def kernel(**inputs):
    from concourse import bass_utils
    nc = _get_nc()
    in_maps = shard_inputs(**inputs)
    res = bass_utils.run_bass_kernel_spmd(nc, in_maps, core_ids=list(range(8)))
    return unshard_outputs(res.results)



# revision 10
# speedup vs baseline: 1.5457x; 1.5457x over previous
"""Causal self-attention Bass/Tile kernel for Trainium2, 8 NeuronCores.

Problem: B=4, T=2048, C=1024, NH=16, HD=64.
  q/k/v = x @ W{q,k,v}; att = softmax(causal(q k^T / 8)); y = (att v) @ Wp

Sharding (8 cores): batch (4-way) x head-group (2-way tensor parallel).
Core c handles batch b=c//2 and global heads g*8..g*8+7 where g=c%2.
Each core computes a partial projection y_part = y_heads_local @ Wp[rows]
(emitted bf16) and the host unshards by summing the two partials per batch
in fp32.

Per-core kernel (all T=2048 tokens, 8 heads, head_dim 64), bf16 matmuls
with fp32 PSUM accumulation:
  A: qT/kT = (x W)^T stored [d, t] (bf16); v stored [t, d] with a ones
     column at d=64.
  B: per query tile j (512 wide) / head pair pr: transposed score tiles
     S^T [s:128, q:512] on PE (two heads on disjoint PE row halves),
     exp(S/8) on ACT (PSUM->SBUF bf16); diagonal s-tiles computed at
     reduced width (q >= 128r) and causally masked by a DVE multiply
     with a precomputed lower-triangle [128,128] mask; PV flipped:
     out[q:128, 0:65] += P_chunk^T^T @ [v|1] (N=65 per accumulation
     step) accumulating y_unnorm and the softmax denominator l in PSUM
     column 64, per (query 128-subtile, head).
  C: per block: l -> 1/l (DVE reciprocal), normalize fused into the
     PSUM->SBUF copy via a stride-0 broadcast multiply -> y_sb [q, u];
     per completed q-tile, XBAR DMA-transpose y_sb -> ylocT [u, t].
  D: y_part[t, c] = sum_u ylocT[u, t] * Wp[u, c] on PE, staged bf16,
     DMA'd out.
Projection (A) and output-projection (D) PE work is interleaved into the
ACT-paced attention stream by virtual engine clocks so the PE queue never
starves (emission order == execution order per engine).
"""

from collections import deque

import numpy as np

B, T, C, NH, HD = 4, 2048, 1024, 16, 64
G = 512          # local head dims per core (8 heads x 64)
P = 128
NT = 4           # q tiles of 512
NT128 = 16       # t tiles of 128
NPAIR = 4        # local head pairs
TT = 512

_CACHE = {}
DEBUG = False


def _build_nc():
    import concourse.tile as tile
    from concourse import bacc, mybir

    f32 = mybir.dt.float32
    bf16 = mybir.dt.bfloat16
    EXP = mybir.ActivationFunctionType.Exp
    MUL = mybir.AluOpType.mult

    nc = bacc.Bacc("TRN2", target_bir_lowering=False, debug=False)

    xT = nc.dram_tensor("xt", [C, T], bf16, kind="ExternalInput")
    wq = nc.dram_tensor("wq", [C, G], bf16, kind="ExternalInput")
    wk = nc.dram_tensor("wk", [C, G], bf16, kind="ExternalInput")
    wv = nc.dram_tensor("wv", [C, G], bf16, kind="ExternalInput")
    wp = nc.dram_tensor("wp", [G, C], bf16, kind="ExternalInput")
    tri = nc.dram_tensor("tri", [P, P], bf16, kind="ExternalInput")
    y = nc.dram_tensor("y", [T, C], bf16, kind="ExternalOutput")
    if DEBUG:
        ydbg = nc.dram_tensor("ydbg", [P, NT128, G], bf16, kind="ExternalOutput")
        qdbg = nc.dram_tensor("qdbg", [P, NPAIR, T], bf16, kind="ExternalOutput")
        kdbg = nc.dram_tensor("kdbg", [P, NPAIR, T], bf16, kind="ExternalOutput")
        vdbg = nc.dram_tensor("vdbg", [P, NT128, 8, 66], bf16, kind="ExternalOutput")
        tdbg = nc.dram_tensor("tdbg", [P, NPAIR, T], bf16, kind="ExternalOutput")
        edbg = nc.dram_tensor("edbg", [P, 8, 2, TT], bf16, kind="ExternalOutput")

    xT_v = xT.rearrange("(co p) t -> p co t", p=P)      # [128, 8, 2048]
    wq_v = wq.rearrange("(co p) g -> p co g", p=P)      # [128, 8, 512]
    wk_v = wk.rearrange("(co p) g -> p co g", p=P)
    wv_v = wv.rearrange("(co p) g -> p co g", p=P)
    wp_v = wp.rearrange("(uo p) c -> p uo c", p=P)      # [128, 4, 1024]
    y_v = y.rearrange("(to p) c -> p to c", p=P)        # [128, 16, 1024]

    with tile.TileContext(nc) as tc:
        with (
            tc.tile_pool(name="singles", bufs=1) as singles,
            tc.tile_pool(name="expst", bufs=2) as epool,
            tc.tile_pool(name="rec", bufs=2) as rpool,
            tc.tile_pool(name="ystage", bufs=3) as ypool,
            tc.tile_pool(name="psA", bufs=2, space="PSUM") as psA,
            tc.tile_pool(name="psS", bufs=2, space="PSUM") as psS,
            tc.tile_pool(name="psY", bufs=1, space="PSUM") as psYp,
        ):
            # ---------------- persistent SBUF tensors ----------------
            x_sb = singles.tile([P, 8, T], bf16, name="x_sb", tag="x_sb")
            wq_sb = singles.tile([P, 8, G], bf16, name="wq_sb", tag="wq_sb")
            wk_sb = singles.tile([P, 8, G], bf16, name="wk_sb", tag="wk_sb")
            wv_sb = singles.tile([P, 8, G], bf16, name="wv_sb", tag="wv_sb")
            wp_sb = singles.tile([P, NPAIR, C], bf16, name="wp_sb", tag="wp_sb")
            tri_sb = singles.tile([P, P], bf16, name="tri_sb", tag="tri_sb")
            qT = singles.tile([P, NPAIR, T], bf16, name="qT", tag="qT")
            kT = singles.tile([P, NPAIR, T], bf16, name="kT", tag="kT")
            v_sb = singles.tile([P, NT128, 8, 66], bf16, name="v_sb", tag="v_sb")
            y_sb = singles.tile([P, NT128, G], bf16, name="y_sb", tag="y_sb")
            ylocT = singles.tile([P, NPAIR, T], bf16, name="ylocT", tag="ylocT")

            nc.vector.memset(v_sb[:, :, :, 64:65], 1.0)

            # ---------------- input DMA (ordered for earliest PE start) ----
            # dg0 slices of wq/wk first so the first qk units unblock early;
            # x in 512-token chunks (the unit A consumes); tri tiny via SWDGE.
            nc.scalar.dma_start(wq_sb[:, :, 0:P], wq_v[:, :, 0:P])
            nc.scalar.dma_start(wk_sb[:, :, 0:P], wk_v[:, :, 0:P])
            nc.gpsimd.dma_start(tri_sb[:], tri[:])
            for jj in range(NT):
                nc.sync.dma_start(
                    x_sb[:, :, jj * TT:(jj + 1) * TT],
                    xT_v[:, :, jj * TT:(jj + 1) * TT])
            nc.scalar.dma_start(wv_sb[:], wv_v[:])
            nc.scalar.dma_start(wq_sb[:, :, P:G], wq_v[:, :, P:G])
            nc.scalar.dma_start(wk_sb[:, :, P:G], wk_v[:, :, P:G])
            nc.gpsimd.dma_start(wp_sb[:], wp_v[:])

            # ---------------- virtual engine clocks ----------------
            clk = {"pe": 0.0, "act": 0.0}
            PEC = 1.0 / 2.4          # ns per output row, ramped PE
            ACTC = 1.0 / 1.2         # ns per free element, ACT
            MARGIN = 2500.0          # keep this much PE work emitted ahead

            # ---------------- A work units (projections) ----------------
            def emit_qk(dg, view):
                w_sb, dstT = ((wq_sb, qT), (wk_sb, kT))[view]

                def f(dg=dg, view=view, w_sb=w_sb, dstT=dstT):
                    for jj in range(NT):
                        ps = psA.tile([P, TT], f32, name="ps_qk", tag="psA")
                        for co in range(8):
                            nc.tensor.matmul(
                                ps[:], w_sb[:, co, dg * P:(dg + 1) * P],
                                x_sb[:, co, jj * TT:(jj + 1) * TT],
                                start=(co == 0), stop=(co == 7))
                        clk["pe"] += 8 * TT * PEC
                        nc.vector.tensor_copy(
                            out=dstT[:, dg, jj * TT:(jj + 1) * TT], in_=ps[:])
                return f

            def emit_qk1(dg, view, jj):
                w_sb, dstT = ((wq_sb, qT), (wk_sb, kT))[view]

                def f(dg=dg, view=view, jj=jj, w_sb=w_sb, dstT=dstT):
                    ps = psA.tile([P, TT], f32, name="ps_qk", tag="psA")
                    for co in range(8):
                        nc.tensor.matmul(
                            ps[:], w_sb[:, co, dg * P:(dg + 1) * P],
                            x_sb[:, co, jj * TT:(jj + 1) * TT],
                            start=(co == 0), stop=(co == 7))
                    clk["pe"] += 8 * TT * PEC
                    nc.vector.tensor_copy(
                        out=dstT[:, dg, jj * TT:(jj + 1) * TT], in_=ps[:])
                return f

            def emit_v(t128):
                def f(t128=t128):
                    ps = psA.tile([P, G], f32, name="ps_v", tag="psA")
                    for co in range(8):
                        nc.tensor.matmul(
                            ps[:], x_sb[:, co, t128 * P:(t128 + 1) * P],
                            wv_sb[:, co, :],
                            start=(co == 0), stop=(co == 7))
                    clk["pe"] += 8 * G * PEC
                    nc.vector.tensor_copy(
                        out=v_sb[:, t128, :, 0:64],
                        in_=ps.rearrange("p (h d) -> p h d", h=8))
                return f

            # wave j of attention requires qT[:, pr, jTT:(j+1)TT] (unit
            # (pr, q, jj=j)), kT up to column (j+1)TT, and v tiles <= 4j+3.
            # Order units by the wave that first needs them.
            awork = deque()
            wave_req = []
            for j in range(NT):
                units = []
                for pr in range(NPAIR):
                    units.append(emit_qk1(pr, 0, j))
                    units.append(emit_qk1(pr, 1, j))
                for t128 in range(4 * j, 4 * j + 4):
                    units.append(emit_v(t128))
                wave_req.append(units)
                awork.extend(units)
            emitted = set()

            def run_unit(u):
                if id(u) in emitted:
                    return
                emitted.add(id(u))
                u()

            dwork = deque()

            def emit_d(t128, cn):
                def f(t128=t128, cn=cn):
                    ps = psA.tile([P, TT], f32, name="ps_y", tag="psA")
                    for uo in range(NPAIR):
                        nc.tensor.matmul(
                            ps[:],
                            ylocT[:, uo, t128 * P:(t128 + 1) * P],
                            wp_sb[:, uo, cn * TT:(cn + 1) * TT],
                            start=(uo == 0), stop=(uo == 3))
                    clk["pe"] += 4 * TT * PEC
                    yst = ypool.tile([P, TT], bf16, name="yst", tag="yst")
                    nc.vector.tensor_copy(out=yst[:], in_=ps[:])
                    nc.sync.dma_start(
                        out=y_v[:, t128, cn * TT:(cn + 1) * TT],
                        in_=yst[:])
                return f

            def filler():
                while clk["pe"] < clk["act"] + MARGIN:
                    if awork:
                        u = awork.popleft()
                        if id(u) in emitted:
                            continue
                        run_unit(u)
                    elif dwork:
                        dwork.popleft()()
                    else:
                        break

            # ---------------- attention block ----------------
            def emit_block(j, pr):
                n_off = 4 * j                      # full-width s-tiles
                expp_lo = epool.tile(
                    [P, 8, 2, TT], bf16, name="expp_lo", tag="expp")
                expp_hi = expp_lo if 4 * (j + 1) <= 8 else epool.tile(
                    [P, 8, 2, TT], bf16, name="expp_hi", tag="expp")
                if DEBUG:
                    nc.vector.memset(expp_lo[:], 0.0)
                    if expp_hi is not expp_lo:
                        nc.vector.memset(expp_hi[:], 0.0)

                def eslot(so):
                    t = expp_lo if so < 8 else expp_hi
                    return t[:, so % 8]

                psY = psYp.tile(
                    [P, 2, 4, 65], f32, name="psY", tag="psY",
                    padded_shape=[P, 2, 4, P])
                # PSUM start zeroes the whole 2KB bank (one bank per hi
                # here): exactly one start per bank (first matmul of the
                # block) and one stop (last matmul, diag r=3 / qq=3).
                bank_started = set()

                def pv_group(so, r):
                    # r is None for full-width tiles; diag tile 4j+r feeds
                    # only query subtiles qq >= r.
                    for qq in range(0 if r is None else r, 4):
                        for hi in range(2):
                            h = 2 * pr + hi
                            st = hi not in bank_started
                            bank_started.add(hi)
                            stop = (r == 3) and (qq == 3)
                            nc.tensor.matmul(
                                psY[:, hi, qq, 0:65],
                                eslot(so)[:, hi, qq * P:(qq + 1) * P],
                                v_sb[:, so, h, 0:65],
                                start=st, stop=stop, skip_group_check=True)
                            clk["pe"] += 65 * PEC

                def scores_step(so, r):
                    off = 0 if r is None else P * r
                    ps_s = psS.tile([P, 2, TT], f32, name="ps_s", tag="psS")
                    for hi in range(2):
                        hp = 64 * hi
                        nc.tensor.matmul(
                            ps_s[:, hi, off:TT],
                            kT[hp:hp + 64, pr, so * P:(so + 1) * P],
                            qT[hp:hp + 64, pr, j * TT + off:(j + 1) * TT],
                            start=True, stop=True)
                    clk["pe"] += 2 * (TT - off) * PEC
                    nc.scalar.activation(
                        out=eslot(so)[:, :, off:TT],
                        in_=ps_s[:, :, off:TT],
                        func=EXP, scale=0.125)
                    clk["act"] += 2 * (TT - off) * ACTC + 190
                    if r is not None:
                        # causal mask on the diagonal 128x128 subtile
                        nc.vector.tensor_tensor(
                            out=eslot(so)[:, :, off:off + P],
                            in0=eslot(so)[:, :, off:off + P],
                            in1=tri_sb.unsqueeze(1).broadcast_to((P, 2, P)),
                            op=MUL)

                steps = [(so, None) for so in range(n_off)]
                steps += [(4 * j + r, r) for r in range(4)]
                prev = None
                for so, r in steps:
                    filler()
                    scores_step(so, r)
                    if prev is not None:
                        pv_group(*prev)
                    prev = (so, r)
                pv_group(*prev)
                if DEBUG and j == 1 and pr == 0:
                    nc.sync.dma_start(edbg[:], expp_lo[:])

                # normalize: 1/l then fused scale on the PSUM->SBUF copy
                rec = rpool.tile([P, 2, 4, 1], f32, name="rec", tag="rec")
                nc.vector.reciprocal(out=rec[:], in_=psY[:, :, :, 64:65])
                ysl = y_sb[:, 4 * j:4 * j + 4, pr * P:(pr + 1) * P]
                nc.vector.tensor_tensor(
                    out=ysl.rearrange("p a (hi d) -> p a hi d", hi=2),
                    in0=psY[:, :, :, 0:64].rearrange("p hi qq d -> p qq hi d"),
                    in1=rec.rearrange("p hi qq x -> p qq hi x")
                        .broadcast_to((P, 4, 2, 64)),
                    op=MUL)

            # ---------------- main schedule ----------------
            for j in range(NT):
                for u in wave_req[j]:
                    run_unit(u)
                for pr in range(NPAIR):
                    emit_block(j, pr)
                # q-tile j fully normalized: transpose + release D work
                for qq in range(4):
                    t128 = 4 * j + qq
                    nc.sync.dma_start_transpose(
                        out=ylocT[:, :, t128 * P:(t128 + 1) * P],
                        in_=y_sb[:, t128, :])
                    dwork.append(emit_d(t128, 0))
                    dwork.append(emit_d(t128, 1))

            while awork or dwork:
                if awork:
                    u = awork.popleft()
                    if id(u) in emitted:
                        continue
                    run_unit(u)
                else:
                    dwork.popleft()()

            if DEBUG:
                nc.sync.dma_start(ydbg[:], y_sb[:])
                nc.sync.dma_start(qdbg[:], qT[:])
                nc.sync.dma_start(kdbg[:], kT[:])
                nc.sync.dma_start(vdbg[:, :, :, 0:65], v_sb[:, :, :, 0:65])
                nc.sync.dma_start(tdbg[:], ylocT[:])

    nc.finalize()
    return nc


def _get_nc():
    if "nc" not in _CACHE:
        _CACHE["nc"] = _build_nc()
    return _CACHE["nc"]


def _tri_array():
    import ml_dtypes
    # tri[s, q'] = 1 where q' >= s (causally valid within a diagonal block)
    return np.triu(np.ones((P, P), np.float32)).astype(ml_dtypes.bfloat16)


def shard_inputs(x, Wq, Wk, Wv, Wp):
    """Build the 8 per-core input maps."""
    import ml_dtypes
    bf = ml_dtypes.bfloat16
    x = np.asarray(x, np.float32)
    Wq, Wk, Wv, Wp = (np.asarray(w, np.float32) for w in (Wq, Wk, Wv, Wp))
    tri = _tri_array()
    in_maps = []
    for c in range(8):
        b, g = c // 2, c % 2
        sl = slice(g * G, (g + 1) * G)
        in_maps.append({
            "xt": np.ascontiguousarray(x[b].T).astype(bf),
            "wq": np.ascontiguousarray(Wq[:, sl]).astype(bf),
            "wk": np.ascontiguousarray(Wk[:, sl]).astype(bf),
            "wv": np.ascontiguousarray(Wv[:, sl]).astype(bf),
            "wp": np.ascontiguousarray(Wp[sl, :]).astype(bf),
            "tri": tri,
        })
    return in_maps


def unshard_outputs(results):
    """results: list of 8 dicts with 'y' [T, C] bf16 partials -> [B, T, C]."""
    out = np.empty((B, T, C), np.float32)
    for b in range(B):
        out[b] = (results[2 * b]["y"].astype(np.float32)
                  + results[2 * b + 1]["y"].astype(np.float32))
    return out


def kernel(**inputs):
    from concourse import bass_utils
    nc = _get_nc()
    in_maps = shard_inputs(**inputs)
    res = bass_utils.run_bass_kernel_spmd(nc, in_maps, core_ids=list(range(8)))
    return unshard_outputs(res.results)


# revision 41
# speedup vs baseline: 1.7220x; 1.1141x over previous
"""Causal self-attention Bass/Tile kernel for Trainium2, 8 NeuronCores.

Problem: B=4, T=2048, C=1024, NH=16, HD=64.
  q/k/v = x @ W{q,k,v}; att = softmax(causal(q k^T / 8)); y = (att v) @ Wp

Sharding (8 cores): batch (4-way) x head-group (2-way tensor parallel).
Core c handles batch b=c//2 and global heads g*8..g*8+7 where g=c%2.
Each core computes a partial projection y_part = y_heads_local @ Wp[rows]
(emitted bf16) and the host unshards by summing the two partials per batch
in fp32.

Per-core kernel (all T=2048 tokens, 8 heads, head_dim 64), bf16 matmuls
with fp32 PSUM accumulation:
  A: qT/kT = (x W)^T stored [d, t] (bf16); v stored [t, d] with a ones
     column at d=64.
  B: per query tile j (512 wide) / head pair pr: transposed score tiles
     S^T [s:128, q:512] on PE (two heads on disjoint PE row halves),
     exp(S/8) on ACT (PSUM->SBUF bf16); diagonal s-tiles computed at
     reduced width (q >= 128r) and causally masked by a DVE multiply
     with a precomputed lower-triangle [128,128] mask; PV flipped:
     out[q:128, 0:65] += P_chunk^T^T @ [v|1] (N=65 per accumulation
     step) accumulating y_unnorm and the softmax denominator l in PSUM
     column 64, per (query 128-subtile, head).
  C: per block: l -> 1/l (DVE reciprocal), normalize fused into the
     PSUM->SBUF copy via a stride-0 broadcast multiply -> y_sb [q, u];
     per completed q-tile, XBAR DMA-transpose y_sb -> ylocT [u, t].
  D: y_part[t, c] = sum_u ylocT[u, t] * Wp[u, c] on PE, staged bf16,
     DMA'd out.
Projection (A) and output-projection (D) PE work is interleaved into the
ACT-paced attention stream by virtual engine clocks so the PE queue never
starves (emission order == execution order per engine).
"""

from collections import deque

import numpy as np

B, T, C, NH, HD = 4, 2048, 1024, 16, 64
G = 512          # local head dims per core (8 heads x 64)
P = 128
NT = 4           # q tiles of 512
NT128 = 16       # t tiles of 128
NPAIR = 4        # local head pairs
TT = 512

_CACHE = {}
DEBUG = False


def _build_nc():
    import concourse.tile as tile
    from concourse import bacc, mybir

    f32 = mybir.dt.float32
    bf16 = mybir.dt.bfloat16
    EXP = mybir.ActivationFunctionType.Exp
    MUL = mybir.AluOpType.mult

    nc = bacc.Bacc("TRN2", target_bir_lowering=False, debug=False)

    xT = nc.dram_tensor("xt", [C, T], bf16, kind="ExternalInput")
    # wq/wk arrive dg-major: [p, dg, co, d] = W[co*128+p, dg*128+d], so the
    # dg0 slice is one contiguous 2KB-row DMA (fast first arrival).
    wq = nc.dram_tensor("wq", [P, NPAIR, 8, P], bf16, kind="ExternalInput")
    wk = nc.dram_tensor("wk", [P, NPAIR, 8, P], bf16, kind="ExternalInput")
    wv = nc.dram_tensor("wv", [C, G], bf16, kind="ExternalInput")
    wp = nc.dram_tensor("wp", [G, C], bf16, kind="ExternalInput")
    tri = nc.dram_tensor("tri", [P, P], bf16, kind="ExternalInput")
    y = nc.dram_tensor("y", [T, C], bf16, kind="ExternalOutput")
    if DEBUG:
        ydbg = nc.dram_tensor("ydbg", [P, NT128, G], bf16, kind="ExternalOutput")
        qdbg = nc.dram_tensor("qdbg", [P, NPAIR, T], bf16, kind="ExternalOutput")
        kdbg = nc.dram_tensor("kdbg", [P, NPAIR, T], bf16, kind="ExternalOutput")
        vdbg = nc.dram_tensor("vdbg", [P, NT128, 8, 66], bf16, kind="ExternalOutput")
        tdbg = nc.dram_tensor("tdbg", [P, NPAIR, T], bf16, kind="ExternalOutput")
        edbg = nc.dram_tensor("edbg", [P, 8, 2, TT], bf16, kind="ExternalOutput")

    xT_v = xT.rearrange("(co p) t -> p co t", p=P)      # [128, 8, 2048]
    wv_v = wv.rearrange("(co p) g -> p co g", p=P)      # [128, 8, 512]
    wp_v = wp.rearrange("(uo p) c -> p uo c", p=P)      # [128, 4, 1024]
    y_v = y.rearrange("(to p) c -> p to c", p=P)        # [128, 16, 1024]

    with tile.TileContext(nc) as tc:
        with (
            tc.tile_pool(name="singles", bufs=1) as singles,
            tc.tile_pool(name="expst", bufs=2) as epool,
            tc.tile_pool(name="rec", bufs=2) as rpool,
            tc.tile_pool(name="ystage", bufs=8) as ypool,
            tc.tile_pool(name="psA", bufs=2, space="PSUM") as psA,
            tc.tile_pool(name="psS", bufs=2, space="PSUM") as psS,
            tc.tile_pool(name="psY", bufs=1, space="PSUM") as psYp,
        ):
            # ---------------- persistent SBUF tensors ----------------
            x_sb = singles.tile([P, 8, T], bf16, name="x_sb", tag="x_sb")
            wq_sb = singles.tile([P, NPAIR, 8, P], bf16, name="wq_sb",
                                 tag="wq_sb")
            wk_sb = singles.tile([P, NPAIR, 8, P], bf16, name="wk_sb",
                                 tag="wk_sb")
            wv_sb = singles.tile([P, 8, G], bf16, name="wv_sb", tag="wv_sb")
            wp_sb = singles.tile([P, NPAIR, C], bf16, name="wp_sb", tag="wp_sb")
            tri_sb = singles.tile([P, P], bf16, name="tri_sb", tag="tri_sb")
            qT = singles.tile([P, NPAIR, T], bf16, name="qT", tag="qT")
            kT = singles.tile([P, NPAIR, T], bf16, name="kT", tag="kT")
            v_sb = singles.tile([P, NT128, 8, 66], bf16, name="v_sb", tag="v_sb")
            y_sb = singles.tile([P, NT128, G], bf16, name="y_sb", tag="y_sb")
            ylocT = singles.tile([P, NPAIR, T], bf16, name="ylocT", tag="ylocT")

            nc.vector.memset(v_sb[:, :, :, 64:65], 1.0)

            # ---------------- input DMA (ordered for earliest PE start) ----
            # One HWDGE queue in exact priority order (a single queue gives
            # full control of DMA_ENGINES ordering); tri/wp ride SWDGE (Pool)
            # in parallel.
            nc.gpsimd.dma_start(tri_sb[:], tri[:])
            nc.gpsimd.dma_start(wp_sb[:], wp_v[:])
            nc.scalar.dma_start(wq_sb[:, 0], wq[:, 0])
            nc.scalar.dma_start(wk_sb[:, 0], wk[:, 0])
            nc.scalar.dma_start(x_sb[:, :, 0:TT], xT_v[:, :, 0:TT])
            nc.scalar.dma_start(wv_sb[:], wv_v[:])
            nc.scalar.dma_start(wq_sb[:, 1:NPAIR], wq[:, 1:NPAIR])
            nc.scalar.dma_start(wk_sb[:, 1:NPAIR], wk[:, 1:NPAIR])
            for jj in range(1, NT):
                nc.scalar.dma_start(
                    x_sb[:, :, jj * TT:(jj + 1) * TT],
                    xT_v[:, :, jj * TT:(jj + 1) * TT])

            # ---------------- virtual engine clocks ----------------
            # clk["pe"]: estimated completion time of all emitted PE work
            # (valid while PE never stalls — which fill_until enforces).
            # exp_done[i]: estimated completion time of the i-th exp.
            clk = {"pe": 0.0, "act": 0.0}
            PEC = 1.0 / 2.4          # ns per output row, ramped PE
            ACTC = 1.0 / 1.2         # ns per free element, ACT
            MARGIN = 2500.0          # keep this much PE work emitted ahead

            # ---------------- A work units (projections) ----------------
            def emit_qk1(dg, view, jj):
                w_sb, dstT = ((wq_sb, qT), (wk_sb, kT))[view]

                def f(dg=dg, view=view, jj=jj, w_sb=w_sb, dstT=dstT):
                    ps = psA.tile([P, TT], f32, name="ps_qk", tag="psA")
                    for co in range(8):
                        nc.tensor.matmul(
                            ps[:], w_sb[:, dg, co, :],
                            x_sb[:, co, jj * TT:(jj + 1) * TT],
                            start=(co == 0), stop=(co == 7))
                    clk["pe"] += 8 * TT * PEC
                    nc.vector.tensor_copy(
                        out=dstT[:, dg, jj * TT:(jj + 1) * TT], in_=ps[:])
                return f

            def emit_v(t128):
                def f(t128=t128):
                    ps = psA.tile([P, G], f32, name="ps_v", tag="psA")
                    for co in range(8):
                        nc.tensor.matmul(
                            ps[:], x_sb[:, co, t128 * P:(t128 + 1) * P],
                            wv_sb[:, co, :],
                            start=(co == 0), stop=(co == 7))
                    clk["pe"] += 8 * G * PEC
                    nc.vector.tensor_copy(
                        out=v_sb[:, t128, :, 0:64],
                        in_=ps.rearrange("p (h d) -> p h d", h=8))
                return f

            # Block (j, pr) requires qT/kT unit (pr, view, jj=j) (k for
            # jj<j covered inductively by earlier blocks) and v tiles
            # <= 4j+3. Force-emit per block; everything else rides filler.
            qk_units = {}
            v_units = {}
            awork = deque()
            for j in range(NT):
                for view in range(2):
                    for pr in range(NPAIR):
                        u = emit_qk1(pr, view, j)
                        qk_units[(pr, view, j)] = u
                        awork.append(u)
                for t128 in range(4 * j, 4 * j + 4):
                    u = emit_v(t128)
                    v_units[t128] = u
                    awork.append(u)
            emitted = set()

            def run_unit(u):
                if id(u) in emitted:
                    return
                emitted.add(id(u))
                u()

            dwork = deque()

            def emit_d(t128):
                def f(t128=t128):
                    yst = ypool.tile([P, 2, TT], bf16, name="yst", tag="yst")
                    for cn in range(2):
                        ps = psA.tile([P, TT], f32, name="ps_y", tag="psA")
                        for uo in range(NPAIR):
                            nc.tensor.matmul(
                                ps[:],
                                ylocT[:, uo, t128 * P:(t128 + 1) * P],
                                wp_sb[:, uo, cn * TT:(cn + 1) * TT],
                                start=(uo == 0), stop=(uo == 3))
                        clk["pe"] += 4 * TT * PEC
                        nc.vector.tensor_copy(out=yst[:, cn, :], in_=ps[:])
                    # drain-phase output DMAs ride the (now idle) ACT queue
                    eng = nc.scalar if clk.get("drain") else nc.sync
                    eng.dma_start(out=y_v[:, t128, :], in_=yst[:])
                return f

            DRESERVE = 4   # D units held back to cover the end-of-kernel drain

            def fill_until(target):
                while clk["pe"] < target:
                    if awork:
                        u = awork.popleft()
                        if id(u) in emitted:
                            continue
                        run_unit(u)
                    elif len(dwork) > DRESERVE:
                        dwork.popleft()()
                    else:
                        break

            def filler():
                fill_until(clk["act"] + MARGIN)

            # ---------------- attention block ----------------
            def emit_block(j, pr, pre_pv=()):
                # diag s-tiles first: their DVE masks land while the
                # off-diag score/exp stream runs, so PV never stalls on a
                # mask. PV groups trail the score stream by 2 steps.
                expp_lo = epool.tile(
                    [P, 8, 2, TT], bf16, name="expp_lo", tag="expp")
                expp_hi = expp_lo if 4 * (j + 1) <= 8 else epool.tile(
                    [P, 8, 2, TT], bf16, name="expp_hi", tag="expp")
                if DEBUG:
                    nc.vector.memset(expp_lo[:], 0.0)
                    if expp_hi is not expp_lo:
                        nc.vector.memset(expp_hi[:], 0.0)

                def eslot(so):
                    t = expp_lo if so < 8 else expp_hi
                    return t[:, so % 8]

                psY = psYp.tile(
                    [P, 2, 4, 65], f32, name="psY", tag="psY",
                    padded_shape=[P, 2, 4, P])
                # PSUM start zeroes the whole 2KB bank (one bank per hi
                # here): exactly one start per bank (first matmul of the
                # block) and one stop (last matmul into it).
                bank_started = set()

                def pv_group(so, r, last):
                    # r is None for full-width tiles; diag tile 4j+r feeds
                    # only query subtiles qq >= r.
                    for qq in range(0 if r is None else r, 4):
                        for hi in range(2):
                            h = 2 * pr + hi
                            st = hi not in bank_started
                            bank_started.add(hi)
                            nc.tensor.matmul(
                                psY[:, hi, qq, 0:65],
                                eslot(so)[:, hi, qq * P:(qq + 1) * P],
                                v_sb[:, so, h, 0:65],
                                start=st, stop=(last and qq == 3),
                                skip_group_check=True)
                            clk["pe"] += 65 * PEC

                def scores_step(so, r):
                    off = 0 if r is None else P * r
                    ps_s = psS.tile([P, 2, TT], f32, name="ps_s", tag="psS")
                    for hi in range(2):
                        hp = 64 * hi
                        nc.tensor.matmul(
                            ps_s[:, hi, off:TT],
                            kT[hp:hp + 64, pr, so * P:(so + 1) * P],
                            qT[hp:hp + 64, pr, j * TT + off:(j + 1) * TT],
                            start=True, stop=True)
                    clk["pe"] += 2 * (TT - off) * PEC
                    nc.scalar.activation(
                        out=eslot(so)[:, :, off:TT],
                        in_=ps_s[:, :, off:TT],
                        func=EXP, scale=0.125)
                    clk["act"] += 2 * (TT - off) * ACTC + 190
                    if r is not None:
                        # causal mask on the diagonal 128x128 subtile
                        nc.vector.tensor_tensor(
                            out=eslot(so)[:, :, off:off + P],
                            in0=eslot(so)[:, :, off:off + P],
                            in1=tri_sb.unsqueeze(1).broadcast_to((P, 2, P)),
                            op=MUL)

                steps = [(4 * j + r, r) for r in range(4)]
                steps += [(so, None) for so in range(4 * j)]
                LAG = 2
                npv = 0

                def next_pv():
                    nonlocal npv
                    if npv == 0:
                        for u in pre_pv:
                            run_unit(u)
                    so, r = steps[npv]
                    npv += 1
                    pv_group(so, r, last=(npv == len(steps)))

                for si, (so, r) in enumerate(steps):
                    filler()
                    scores_step(so, r)
                    if si >= LAG:
                        filler()
                        next_pv()
                while npv < len(steps):
                    filler()
                    next_pv()
                if DEBUG and j == 1 and pr == 0:
                    nc.sync.dma_start(edbg[:], expp_lo[:])

                # normalize: 1/l then fused scale on the PSUM->SBUF copy
                rec = rpool.tile([P, 2, 4, 1], f32, name="rec", tag="rec")
                nc.vector.reciprocal(out=rec[:], in_=psY[:, :, :, 64:65])
                ysl = y_sb[:, 4 * j:4 * j + 4, pr * P:(pr + 1) * P]
                nc.vector.tensor_tensor(
                    out=ysl.rearrange("p a (hi d) -> p a hi d", hi=2),
                    in0=psY[:, :, :, 0:64].rearrange("p hi qq d -> p qq hi d"),
                    in1=rec.rearrange("p hi qq x -> p qq hi x")
                        .broadcast_to((P, 4, 2, 64)),
                    op=MUL)

            # ---------------- main schedule ----------------
            for j in range(NT):
                for pr in range(NPAIR):
                    run_unit(qk_units[(pr, 0, j)])
                    run_unit(qk_units[(pr, 1, j)])
                    pre = ([v_units[t128] for t128 in range(4 * j, 4 * j + 4)]
                           if pr == 0 else ())
                    emit_block(j, pr, pre_pv=pre)
                # q-tile j fully normalized: transpose + release D work
                # last wave's transposes ride the ACT queue (idle by then)
                # so the sync queue stays free for the output DMAs
                teng = nc.scalar if j == NT - 1 else nc.sync
                for qq in range(4):
                    t128 = 4 * j + qq
                    teng.dma_start_transpose(
                        out=ylocT[:, :, t128 * P:(t128 + 1) * P],
                        in_=y_sb[:, t128, :])
                    dwork.append(emit_d(t128))

            clk["drain"] = True
            while awork or dwork:
                if awork:
                    u = awork.popleft()
                    if id(u) in emitted:
                        continue
                    run_unit(u)
                else:
                    dwork.popleft()()

            if DEBUG:
                nc.sync.dma_start(ydbg[:], y_sb[:])
                nc.sync.dma_start(qdbg[:], qT[:])
                nc.sync.dma_start(kdbg[:], kT[:])
                nc.sync.dma_start(vdbg[:, :, :, 0:65], v_sb[:, :, :, 0:65])
                nc.sync.dma_start(tdbg[:], ylocT[:])

    nc.finalize()
    return nc


def _get_nc():
    if "nc" not in _CACHE:
        _CACHE["nc"] = _build_nc()
    return _CACHE["nc"]


def _tri_array():
    import ml_dtypes
    # tri[s, q'] = 1 where q' >= s (causally valid within a diagonal block)
    return np.triu(np.ones((P, P), np.float32)).astype(ml_dtypes.bfloat16)


def shard_inputs(x, Wq, Wk, Wv, Wp):
    """Build the 8 per-core input maps."""
    import ml_dtypes
    bf = ml_dtypes.bfloat16
    x = np.asarray(x, np.float32)
    Wq, Wk, Wv, Wp = (np.asarray(w, np.float32) for w in (Wq, Wk, Wv, Wp))
    tri = _tri_array()
    in_maps = []
    def dg_major(w):
        # [p, dg, co, d] = w[co*128+p, dg*128+d]
        return np.ascontiguousarray(
            w.reshape(8, P, NPAIR, P).transpose(1, 2, 0, 3)).astype(bf)

    for c in range(8):
        b, g = c // 2, c % 2
        sl = slice(g * G, (g + 1) * G)
        in_maps.append({
            "xt": np.ascontiguousarray(x[b].T).astype(bf),
            "wq": dg_major(Wq[:, sl]),
            "wk": dg_major(Wk[:, sl]),
            "wv": np.ascontiguousarray(Wv[:, sl]).astype(bf),
            "wp": np.ascontiguousarray(Wp[sl, :]).astype(bf),
            "tri": tri,
        })
    return in_maps


def unshard_outputs(results):
    """results: list of 8 dicts with 'y' [T, C] bf16 partials -> [B, T, C]."""
    out = np.empty((B, T, C), np.float32)
    for b in range(B):
        out[b] = (results[2 * b]["y"].astype(np.float32)
                  + results[2 * b + 1]["y"].astype(np.float32))
    return out


def kernel(**inputs):
    from concourse import bass_utils
    nc = _get_nc()
    in_maps = shard_inputs(**inputs)
    res = bass_utils.run_bass_kernel_spmd(nc, in_maps, core_ids=list(range(8)))
    return unshard_outputs(res.results)


# revision 52
# speedup vs baseline: 1.7915x; 1.0404x over previous
"""Causal self-attention Bass/Tile kernel for Trainium2, 8 NeuronCores.

Problem: B=4, T=2048, C=1024, NH=16, HD=64.
  q/k/v = x @ W{q,k,v}; att = softmax(causal(q k^T / 8)); y = (att v) @ Wp

Sharding (8 cores): batch (4-way) x head-group (2-way tensor parallel).
Core c handles batch b=c//2 and global heads g*8..g*8+7 where g=c%2.
Each core computes a partial projection y_part = y_heads_local @ Wp[rows]
(emitted bf16) and the host unshards by summing the two partials per batch
in fp32.

Per-core kernel (all T=2048 tokens, 8 heads, head_dim 64), bf16 matmuls
with fp32 PSUM accumulation:
  A: qT/kT = (x W)^T stored [d, t] (bf16); v stored [t, d] with a ones
     column at d=64.
  B: per query tile j (512 wide) / head pair pr: transposed score tiles
     S^T [s:128, q:512] on PE (two heads on disjoint PE row halves),
     exp(S/8) on ACT (PSUM->SBUF bf16); diagonal s-tiles computed at
     reduced width (q >= 128r) and causally masked by a DVE multiply
     with a precomputed lower-triangle [128,128] mask; PV flipped:
     out[q:128, 0:65] += P_chunk^T^T @ [v|1] (N=65 per accumulation
     step) accumulating y_unnorm and the softmax denominator l in PSUM
     column 64, per (query 128-subtile, head).
  C: per block: l -> 1/l (DVE reciprocal), normalize fused into the
     PSUM->SBUF copy via a stride-0 broadcast multiply -> y_sb [q, u];
     per completed q-tile, XBAR DMA-transpose y_sb -> ylocT [u, t].
  D: y_part[t, c] = sum_u ylocT[u, t] * Wp[u, c] on PE, staged bf16,
     DMA'd out.
Projection (A) and output-projection (D) PE work is interleaved into the
ACT-paced attention stream by virtual engine clocks so the PE queue never
starves (emission order == execution order per engine).
"""

from collections import deque

import numpy as np

B, T, C, NH, HD = 4, 2048, 1024, 16, 64
G = 512          # local head dims per core (8 heads x 64)
P = 128
NT = 4           # q tiles of 512
NT128 = 16       # t tiles of 128
NPAIR = 4        # local head pairs
TT = 512

_CACHE = {}
DEBUG = False


def _build_nc():
    import concourse.tile as tile
    from concourse import bacc, mybir

    f32 = mybir.dt.float32
    bf16 = mybir.dt.bfloat16
    EXP = mybir.ActivationFunctionType.Exp
    MUL = mybir.AluOpType.mult

    nc = bacc.Bacc("TRN2", target_bir_lowering=False, debug=False)

    xT = nc.dram_tensor("xt", [C, T], bf16, kind="ExternalInput")
    # wq/wk arrive dg-major: [p, dg, co, d] = W[co*128+p, dg*128+d], so the
    # dg0 slice is one contiguous 2KB-row DMA (fast first arrival).
    wq = nc.dram_tensor("wq", [P, NPAIR, 8, P], bf16, kind="ExternalInput")
    wk = nc.dram_tensor("wk", [P, NPAIR, 8, P], bf16, kind="ExternalInput")
    wv = nc.dram_tensor("wv", [C, G], bf16, kind="ExternalInput")
    wp = nc.dram_tensor("wp", [G, C], bf16, kind="ExternalInput")
    tri = nc.dram_tensor("tri", [P, P], bf16, kind="ExternalInput")
    y = nc.dram_tensor("y", [T, C], bf16, kind="ExternalOutput")
    if DEBUG:
        ydbg = nc.dram_tensor("ydbg", [P, NT128, G], bf16, kind="ExternalOutput")
        qdbg = nc.dram_tensor("qdbg", [P, NPAIR, T], bf16, kind="ExternalOutput")
        kdbg = nc.dram_tensor("kdbg", [P, NPAIR, T], bf16, kind="ExternalOutput")
        vdbg = nc.dram_tensor("vdbg", [P, NT128, 8, 66], bf16, kind="ExternalOutput")
        tdbg = nc.dram_tensor("tdbg", [P, NPAIR, T], bf16, kind="ExternalOutput")
        edbg = nc.dram_tensor("edbg", [P, 8, 2, TT], bf16, kind="ExternalOutput")

    xT_v = xT.rearrange("(co p) t -> p co t", p=P)      # [128, 8, 2048]
    wv_v = wv.rearrange("(co p) g -> p co g", p=P)      # [128, 8, 512]
    wp_v = wp.rearrange("(uo p) c -> p uo c", p=P)      # [128, 4, 1024]
    y_v = y.rearrange("(to p) c -> p to c", p=P)        # [128, 16, 1024]

    with tile.TileContext(nc) as tc:
        with (
            tc.tile_pool(name="singles", bufs=1) as singles,
            tc.tile_pool(name="expst", bufs=2) as epool,
            tc.tile_pool(name="rec", bufs=2) as rpool,
            tc.tile_pool(name="ystage", bufs=8) as ypool,
            tc.tile_pool(name="psA", bufs=2, space="PSUM") as psA,
            tc.tile_pool(name="psS", bufs=2, space="PSUM") as psS,
            tc.tile_pool(name="psY", bufs=1, space="PSUM") as psYp,
        ):
            # ---------------- persistent SBUF tensors ----------------
            x_sb = singles.tile([P, 8, T], bf16, name="x_sb", tag="x_sb")
            wq_sb = singles.tile([P, NPAIR, 8, P], bf16, name="wq_sb",
                                 tag="wq_sb")
            wk_sb = singles.tile([P, NPAIR, 8, P], bf16, name="wk_sb",
                                 tag="wk_sb")
            wv_sb = singles.tile([P, 8, G], bf16, name="wv_sb", tag="wv_sb")
            wp_sb = singles.tile([P, NPAIR, C], bf16, name="wp_sb", tag="wp_sb")
            tri_sb = singles.tile([P, P], bf16, name="tri_sb", tag="tri_sb")
            qT = singles.tile([P, NPAIR, T], bf16, name="qT", tag="qT")
            kT = singles.tile([P, NPAIR, T], bf16, name="kT", tag="kT")
            v_sb = singles.tile([P, NT128, 8, 66], bf16, name="v_sb", tag="v_sb")
            y_sb = singles.tile([P, NT128, G], bf16, name="y_sb", tag="y_sb")
            ylocT = singles.tile([P, NPAIR, T], bf16, name="ylocT", tag="ylocT")

            nc.vector.memset(v_sb[:, :, :, 64:65], 1.0)

            # ---------------- input DMA (ordered for earliest PE start) ----
            # One HWDGE queue in exact priority order (a single queue gives
            # full control of DMA_ENGINES ordering); tri/wp ride SWDGE (Pool)
            # in parallel.
            nc.gpsimd.dma_start(tri_sb[:], tri[:])
            nc.scalar.dma_start(wq_sb[:, 0], wq[:, 0])
            nc.scalar.dma_start(wk_sb[:, 0], wk[:, 0])
            # first x chunk in co-quarters so the first qk unit's matmuls
            # start as soon as co0-1 land rather than after the full chunk
            for cq in range(4):
                nc.scalar.dma_start(
                    x_sb[:, 2 * cq:2 * cq + 2, 0:TT],
                    xT_v[:, 2 * cq:2 * cq + 2, 0:TT])
            nc.scalar.dma_start(wv_sb[:], wv_v[:])
            nc.scalar.dma_start(wq_sb[:, 1:NPAIR], wq[:, 1:NPAIR])
            nc.scalar.dma_start(wk_sb[:, 1:NPAIR], wk[:, 1:NPAIR])
            for jj in range(1, NT):
                nc.scalar.dma_start(
                    x_sb[:, :, jj * TT:(jj + 1) * TT],
                    xT_v[:, :, jj * TT:(jj + 1) * TT])
            # wp is first needed by D units (~40us in); last in the scalar
            # stream keeps it clear of the startup-critical loads
            nc.scalar.dma_start(wp_sb[:], wp_v[:])

            # ---------------- virtual engine clocks ----------------
            # clk["pe"]: estimated completion time of all emitted PE work
            # (valid while PE never stalls — which fill_until enforces).
            # exp_done[i]: estimated completion time of the i-th exp.
            clk = {"pe": 0.0, "act": 0.0}
            PEC = 1.0 / 2.4          # ns per output row, ramped PE
            ACTC = 1.0 / 1.2         # ns per free element, ACT
            MARGIN = 2500.0          # keep this much PE work emitted ahead

            # ---------------- A work units (projections) ----------------
            # Units are split into two quanta (4 contraction steps each) so
            # the filler can interleave at ~850ns granularity. Quanta of one
            # unit share a psA tile via a cell; FIFO fill order plus
            # whole-unit requirements keep at most one unit half-open, so
            # psA slot recycling stays deadlock-free.
            def emit_qk1(dg, view, jj):
                w_sb, dstT = ((wq_sb, qT), (wk_sb, kT))[view]
                cell = {}

                def fa(dg=dg, w_sb=w_sb, jj=jj):
                    cell["ps"] = psA.tile([P, TT], f32, name="ps_qk",
                                          tag="psA")
                    for co in range(4):
                        nc.tensor.matmul(
                            cell["ps"][:], w_sb[:, dg, co, :],
                            x_sb[:, co, jj * TT:(jj + 1) * TT],
                            start=(co == 0), stop=False)
                    clk["pe"] += 4 * TT * PEC

                def fb(dg=dg, w_sb=w_sb, jj=jj, dstT=dstT):
                    ps = cell.pop("ps")
                    for co in range(4, 8):
                        nc.tensor.matmul(
                            ps[:], w_sb[:, dg, co, :],
                            x_sb[:, co, jj * TT:(jj + 1) * TT],
                            start=False, stop=(co == 7))
                    clk["pe"] += 4 * TT * PEC
                    nc.vector.tensor_copy(
                        out=dstT[:, dg, jj * TT:(jj + 1) * TT], in_=ps[:])
                return [fa, fb]

            def emit_v(t128):
                cell = {}

                def fa(t128=t128):
                    cell["ps"] = psA.tile([P, G], f32, name="ps_v", tag="psA")
                    for co in range(4):
                        nc.tensor.matmul(
                            cell["ps"][:],
                            x_sb[:, co, t128 * P:(t128 + 1) * P],
                            wv_sb[:, co, :],
                            start=(co == 0), stop=False)
                    clk["pe"] += 4 * G * PEC

                def fb(t128=t128):
                    ps = cell.pop("ps")
                    for co in range(4, 8):
                        nc.tensor.matmul(
                            ps[:], x_sb[:, co, t128 * P:(t128 + 1) * P],
                            wv_sb[:, co, :],
                            start=False, stop=(co == 7))
                    clk["pe"] += 4 * G * PEC
                    nc.vector.tensor_copy(
                        out=v_sb[:, t128, :, 0:64],
                        in_=ps.rearrange("p (h d) -> p h d", h=8))
                return [fa, fb]

            # Block (j, pr) requires qT/kT unit (pr, view, jj=j) (k for
            # jj<j covered inductively by earlier blocks) and v tiles
            # <= 4j+3. Force-emit per block; everything else rides filler.
            qk_units = {}
            v_units = {}
            awork = deque()
            for j in range(NT):
                for view in range(2):
                    for pr in range(NPAIR):
                        u = emit_qk1(pr, view, j)
                        qk_units[(pr, view, j)] = u
                        awork.extend(u)
                for t128 in range(4 * j, 4 * j + 4):
                    u = emit_v(t128)
                    v_units[t128] = u
                    awork.extend(u)
            emitted = set()

            def run_quantum(q):
                if id(q) in emitted:
                    return
                emitted.add(id(q))
                q()

            def run_unit(u):
                for q in u:
                    run_quantum(q)

            dwork = deque()

            def emit_d(t128):
                cell = {}

                def one_cn(cn, t128=t128):
                    ps = psA.tile([P, TT], f32, name="ps_y", tag="psA")
                    for uo in range(NPAIR):
                        nc.tensor.matmul(
                            ps[:],
                            ylocT[:, uo, t128 * P:(t128 + 1) * P],
                            wp_sb[:, uo, cn * TT:(cn + 1) * TT],
                            start=(uo == 0), stop=(uo == 3))
                    clk["pe"] += 4 * TT * PEC
                    nc.vector.tensor_copy(out=cell["yst"][:, cn, :], in_=ps[:])

                def fa(t128=t128):
                    cell["yst"] = ypool.tile([P, 2, TT], bf16, name="yst",
                                             tag="yst")
                    one_cn(0)

                def fb(t128=t128):
                    one_cn(1)
                    # drain-phase output DMAs ride the (now idle) ACT queue
                    eng = nc.scalar if clk.get("drain") else nc.sync
                    eng.dma_start(out=y_v[:, t128, :], in_=cell.pop("yst")[:])
                return [fa, fb]

            DRESERVE = 8   # D quanta held back to cover the end-of-kernel drain

            def fill_until(target):
                while clk["pe"] < target:
                    if awork:
                        q = awork.popleft()
                        if id(q) in emitted:
                            continue
                        run_quantum(q)
                    elif len(dwork) > DRESERVE:
                        run_quantum(dwork.popleft())
                    else:
                        break

            def filler():
                fill_until(clk["act"] + MARGIN)

            # ---------------- attention block ----------------
            def emit_block(j, pr, pre_pv=()):
                # diag s-tiles first: their DVE masks land while the
                # off-diag score/exp stream runs, so PV never stalls on a
                # mask. PV groups trail the score stream by 2 steps.
                expp_lo = epool.tile(
                    [P, 8, 2, TT], bf16, name="expp_lo", tag="expp")
                expp_hi = expp_lo if 4 * (j + 1) <= 8 else epool.tile(
                    [P, 8, 2, TT], bf16, name="expp_hi", tag="expp")
                if DEBUG:
                    nc.vector.memset(expp_lo[:], 0.0)
                    if expp_hi is not expp_lo:
                        nc.vector.memset(expp_hi[:], 0.0)

                def eslot(so):
                    t = expp_lo if so < 8 else expp_hi
                    return t[:, so % 8]

                psY = psYp.tile(
                    [P, 2, 4, 65], f32, name="psY", tag="psY",
                    padded_shape=[P, 2, 4, P])
                # PSUM start zeroes the whole 2KB bank (one bank per hi
                # here): exactly one start per bank (first matmul of the
                # block) and one stop (last matmul into it).
                bank_started = set()

                def pv_group(so, r, last):
                    # r is None for full-width tiles; diag tile 4j+r feeds
                    # only query subtiles qq >= r.
                    for qq in range(0 if r is None else r, 4):
                        for hi in range(2):
                            h = 2 * pr + hi
                            st = hi not in bank_started
                            bank_started.add(hi)
                            nc.tensor.matmul(
                                psY[:, hi, qq, 0:65],
                                eslot(so)[:, hi, qq * P:(qq + 1) * P],
                                v_sb[:, so, h, 0:65],
                                start=st, stop=(last and qq == 3),
                                skip_group_check=True)
                            clk["pe"] += 65 * PEC

                def scores_step(so, r):
                    off = 0 if r is None else P * r
                    ps_s = psS.tile([P, 2, TT], f32, name="ps_s", tag="psS")
                    for hi in range(2):
                        hp = 64 * hi
                        nc.tensor.matmul(
                            ps_s[:, hi, off:TT],
                            kT[hp:hp + 64, pr, so * P:(so + 1) * P],
                            qT[hp:hp + 64, pr, j * TT + off:(j + 1) * TT],
                            start=True, stop=True)
                    clk["pe"] += 2 * (TT - off) * PEC
                    nc.scalar.activation(
                        out=eslot(so)[:, :, off:TT],
                        in_=ps_s[:, :, off:TT],
                        func=EXP, scale=0.125)
                    clk["act"] += 2 * (TT - off) * ACTC + 190
                    if r is not None:
                        # causal mask on the diagonal 128x128 subtile
                        nc.vector.tensor_tensor(
                            out=eslot(so)[:, :, off:off + P],
                            in0=eslot(so)[:, :, off:off + P],
                            in1=tri_sb.unsqueeze(1).broadcast_to((P, 2, P)),
                            op=MUL)

                steps = [(4 * j + r, r) for r in range(4)]
                steps += [(so, None) for so in range(4 * j)]
                LAG = 2
                npv = 0

                def next_pv():
                    nonlocal npv
                    if npv == 0:
                        for u in pre_pv:
                            run_unit(u)
                    so, r = steps[npv]
                    npv += 1
                    pv_group(so, r, last=(npv == len(steps)))

                for si, (so, r) in enumerate(steps):
                    filler()
                    scores_step(so, r)
                    if si >= LAG:
                        filler()
                        next_pv()
                while npv < len(steps):
                    filler()
                    next_pv()
                if DEBUG and j == 1 and pr == 0:
                    nc.sync.dma_start(edbg[:], expp_lo[:])

                # normalize: 1/l then fused scale on the PSUM->SBUF copy
                rec = rpool.tile([P, 2, 4, 1], f32, name="rec", tag="rec")
                nc.vector.reciprocal(out=rec[:], in_=psY[:, :, :, 64:65])
                ysl = y_sb[:, 4 * j:4 * j + 4, pr * P:(pr + 1) * P]
                nc.vector.tensor_tensor(
                    out=ysl.rearrange("p a (hi d) -> p a hi d", hi=2),
                    in0=psY[:, :, :, 0:64].rearrange("p hi qq d -> p qq hi d"),
                    in1=rec.rearrange("p hi qq x -> p qq hi x")
                        .broadcast_to((P, 4, 2, 64)),
                    op=MUL)

            # ---------------- main schedule ----------------
            for j in range(NT):
                for pr in range(NPAIR):
                    run_unit(qk_units[(pr, 0, j)])
                    run_unit(qk_units[(pr, 1, j)])
                    pre = ([v_units[t128] for t128 in range(4 * j, 4 * j + 4)]
                           if pr == 0 else ())
                    emit_block(j, pr, pre_pv=pre)
                # q-tile j fully normalized: transpose + release D work
                # last wave's transposes ride the ACT queue (idle by then)
                # so the sync queue stays free for the output DMAs
                teng = nc.scalar if j == NT - 1 else nc.sync
                for qq in range(4):
                    t128 = 4 * j + qq
                    teng.dma_start_transpose(
                        out=ylocT[:, :, t128 * P:(t128 + 1) * P],
                        in_=y_sb[:, t128, :])
                    dwork.extend(emit_d(t128))

            clk["drain"] = True
            while awork or dwork:
                q = awork.popleft() if awork else dwork.popleft()
                run_quantum(q)

            if DEBUG:
                nc.sync.dma_start(ydbg[:], y_sb[:])
                nc.sync.dma_start(qdbg[:], qT[:])
                nc.sync.dma_start(kdbg[:], kT[:])
                nc.sync.dma_start(vdbg[:, :, :, 0:65], v_sb[:, :, :, 0:65])
                nc.sync.dma_start(tdbg[:], ylocT[:])

    nc.finalize()
    return nc


def _get_nc():
    if "nc" not in _CACHE:
        _CACHE["nc"] = _build_nc()
    return _CACHE["nc"]


def _tri_array():
    import ml_dtypes
    # tri[s, q'] = 1 where q' >= s (causally valid within a diagonal block)
    return np.triu(np.ones((P, P), np.float32)).astype(ml_dtypes.bfloat16)


def shard_inputs(x, Wq, Wk, Wv, Wp):
    """Build the 8 per-core input maps."""
    import ml_dtypes
    bf = ml_dtypes.bfloat16
    x = np.asarray(x, np.float32)
    Wq, Wk, Wv, Wp = (np.asarray(w, np.float32) for w in (Wq, Wk, Wv, Wp))
    tri = _tri_array()
    in_maps = []
    def dg_major(w):
        # [p, dg, co, d] = w[co*128+p, dg*128+d]
        return np.ascontiguousarray(
            w.reshape(8, P, NPAIR, P).transpose(1, 2, 0, 3)).astype(bf)

    for c in range(8):
        b, g = c // 2, c % 2
        sl = slice(g * G, (g + 1) * G)
        in_maps.append({
            "xt": np.ascontiguousarray(x[b].T).astype(bf),
            "wq": dg_major(Wq[:, sl]),
            "wk": dg_major(Wk[:, sl]),
            "wv": np.ascontiguousarray(Wv[:, sl]).astype(bf),
            "wp": np.ascontiguousarray(Wp[sl, :]).astype(bf),
            "tri": tri,
        })
    return in_maps


def unshard_outputs(results):
    """results: list of 8 dicts with 'y' [T, C] bf16 partials -> [B, T, C]."""
    out = np.empty((B, T, C), np.float32)
    for b in range(B):
        out[b] = (results[2 * b]["y"].astype(np.float32)
                  + results[2 * b + 1]["y"].astype(np.float32))
    return out


def kernel(**inputs):
    from concourse import bass_utils
    nc = _get_nc()
    in_maps = shard_inputs(**inputs)
    res = bass_utils.run_bass_kernel_spmd(nc, in_maps, core_ids=list(range(8)))
    return unshard_outputs(res.results)
